# revision 33
# baseline (speedup 1.0000x reference)
"""Trainium2 Bass kernel for nn_Attention_48799418417201.

Multi-head attention (B=8, S=1024, E=768, H=12, D=64) with LoRA (R=16) on the
QKV projections. Data-parallel over batch: one batch element per NeuronCore,
8 cores.

Layout strategy (per core):
  - Host passes x^T [E, S] per input (q/k/v), plus pre-transposed weights, all
    fp16. The 1/sqrt(D) scaling is folded into Wq/bq/lora_b[q] on the host.
  - Projections produce Q^T, K^T [E, S] (head-major partitions) and V_aug
    [S, 13*65] (natural, 65 columns per head: 64 V columns + a ones column),
    each with the LoRA delta accumulated into the same PSUM group.
  - Scores are computed transposed: S^T[j, i] = sum_d K^T[d,j] Q^T[d,i], so
    softmax's sum runs over the partition axis -- the ones column in V_aug
    makes the PV matmul emit the softmax denominator Z into PSUM row 64 for
    free (M=65 streams the same cycles as M=64). exp() runs on ScalarE with
    no max-subtraction (scores are bounded ~[-2, 2] for these input scales).
  - PV produces O^T [E, S] directly (V is the stationary operand), which is
    exactly the layout the output projection needs as its stationary side;
    the kernel contains no on-device transposes at all.
  - Head-pair software pipeline: scores/exp for pair t overlap PV and the
    Z-reciprocal/normalize chain for pair t-1, so the output projection
    starts with no serial normalization tail.
"""

import numpy as np
from contextlib import ExitStack

import concourse.bass as bass
import concourse.bacc as bacc
import concourse.tile as tile
from concourse import mybir
from concourse.bass_utils import run_bass_kernel_spmd

P = 128
S = 1024  # sequence length
E = 768  # embedding
H = 12  # heads
D = 64  # head dim
R = 16  # lora rank
NT = E // P  # 6 n-tiles (also e-tiles) per 768-wide dim
MC = S // 512  # 2 moving-chunks of 512 along sequence
MS = S // P  # 8 sequence subtiles of 128
JT = S // P  # 8 j-tiles (key blocks)
IC = S // 512  # 2 i-chunks (query blocks of 512)
VW = D + 1  # 65 columns per head in V_aug

F16 = mybir.dt.float16
F32 = mybir.dt.float32


def build_nc():
    nc = bacc.Bacc("TRN2", target_bir_lowering=False, debug=False, num_devices=8)

    xT = {
        name: nc.dram_tensor(f"x{name}T", [E, S], F16, kind="ExternalInput")
        for name in ("q", "k", "v")
    }
    wT_d = nc.dram_tensor("wT", [E, 3 * E], F16, kind="ExternalInput")
    woT_d = nc.dram_tensor("woT", [E, E], F16, kind="ExternalInput")
    laT_d = nc.dram_tensor("laT", [E, R], F16, kind="ExternalInput")
    lbT_d = nc.dram_tensor("lbT", [R, 3 * E], F16, kind="ExternalInput")
    bqk_d = nc.dram_tensor("bqk", [P, 2 * NT], F32, kind="ExternalInput")
    bv_d = nc.dram_tensor("bv", [E], F32, kind="ExternalInput")
    ob_d = nc.dram_tensor("ob", [E], F32, kind="ExternalInput")
    out_d = nc.dram_tensor("out", [S, E], F32, kind="ExternalOutput")

    with tile.TileContext(nc) as tc, ExitStack() as perm:
        pp = perm.enter_context(tc.tile_pool(name="perm", bufs=1))

        QT = [pp.tile([P, S], F16, name=f"QT{t}", tag=f"QT{t}") for t in range(NT)]
        KT = [pp.tile([P, S], F16, name=f"KT{t}", tag=f"KT{t}") for t in range(NT)]
        Va = [pp.tile([P, H * VW], F16, name=f"Va{m}", tag=f"Va{m}") for m in range(MS)]
        OTu = [pp.tile([P, S], F16, name=f"OTu{t}", tag=f"OTu{t}") for t in range(NT)]
        OTn = [pp.tile([P, S], F16, name=f"OTn{t}", tag=f"OTn{t}") for t in range(NT)]

        sT = {
            n: pp.tile([R, S], F16, name=f"sT{n}", tag=f"sT{n}")
            for n in ("q", "k", "v")
        }
        woT = [pp.tile([P, E], F16, name=f"woT{t}", tag=f"woT{t}") for t in range(NT)]
        bqk = pp.tile([P, 2 * NT], F32, name="bqk", tag="bqk")
        bv_sb = pp.tile([P, E], F32, name="bv_sb", tag="bv_sb")
        ob_sb = pp.tile([P, E], F32, name="ob_sb", tag="ob_sb")
        zbias = pp.tile([P, 1], F32, name="zbias", tag="zbias")

        nc.vector.memset(zbias[:], 0.0)
        nc.gpsimd.dma_start(bqk[:], bqk_d.ap()[:])

        # HAM warm-up: ~3.5us of dummy matmuls while the first input DMAs are
        # in flight (PE is otherwise idle), so real matmuls start at 2.4GHz.
        warm = pp.tile([P, 512], F16, name="warm", tag="warm")
        nc.vector.memset(warm[:], 0.0)

        # ---------------- pools ----------------
        # PSUM bank budget (8 banks total): ppsum 3 + spsum 1 + stp 2x2 = 8.
        # PV accumulators share ppsum's "acc" tag slots; the output projection
        # runs after all of these release and uses op 4x2 banks alone.
        # Allocation order is chosen so releases can be strictly LIFO.
        ppsum = tc.alloc_tile_pool(name="ppsum", bufs=3, space="PSUM")
        spsum = tc.alloc_tile_pool(name="spsum", bufs=1, space="PSUM")
        wpv = tc.alloc_tile_pool(name="wpv", bufs=1)
        xp = tc.alloc_tile_pool(name="xp", bufs=2)
        stp = tc.alloc_tile_pool(name="stp", bufs=2, space="PSUM")
        ep = tc.alloc_tile_pool(name="ep", bufs=26)
        sgp = tc.alloc_tile_pool(name="sgp", bufs=2)
        zbp = tc.alloc_tile_pool(name="zbp", bufs=2)
        zsp = tc.alloc_tile_pool(name="zsp", bufs=1)
        dpool = tc.alloc_tile_pool(name="dpool", bufs=1, space="DRAM")
        wqk = tc.alloc_tile_pool(name="wqk", bufs=1)
        zdram = dpool.tile([H, S], F32, name="zdram", tag="zdram")

        wacc = ppsum.tile([P, 512], F32, name="wacc", tag="acc")
        for _ in range(16):
            nc.tensor.matmul(wacc[:], warm[:, 0:128], warm[:], start=True, stop=True)

        lat = wpv.tile([P, NT, R], F16, name="lat", tag="lat")
        lbt = wpv.tile([R, 3 * E], F16, name="lbt", tag="lbt")
        for k in range(NT):
            nc.gpsimd.dma_start(lat[:, k, :], laT_d.ap()[k * P : (k + 1) * P, :])
        nc.gpsimd.dma_start(lbt[:], lbT_d.ap()[:])
        wreg = {}
        for name in ("q", "k"):
            wreg[name] = [
                wqk.tile([P, E], F16, name=f"w{name}{k}", tag=f"w{name}{k}")
                for k in range(NT)
            ]
        wreg["v"] = [
            wpv.tile([P, E], F16, name=f"wv{k}", tag=f"wv{k}") for k in range(NT)
        ]

        def emit_proj_qk(name, after_n=None):
            noff = (0 if name == "q" else E)
            dest = QT if name == "q" else KT
            bcol = 0 if name == "q" else NT
            for m in range(MC):
                msl = slice(m * 512, (m + 1) * 512)
                xc = xp.tile([P, NT, 512], F16, name=f"xc_{name}{m}", tag="xc")
                for k in range(NT):
                    nc.sync.dma_start(
                        xc[:, k, :], xT[name].ap()[k * P : (k + 1) * P, msl]
                    )
                if m == 0:
                    for k in range(NT):
                        nc.sync.dma_start(
                            wreg[name][k][:],
                            wT_d.ap()[k * P : (k + 1) * P, noff : noff + E],
                        )
                sp = spsum.tile([R, 512], F32, name=f"sp_{name}{m}", tag="sp")
                for k in range(NT):
                    nc.tensor.matmul(
                        sp[:], lat[:, k, :], xc[:, k, :],
                        start=(k == 0), stop=(k == NT - 1),
                    )
                nc.vector.tensor_copy(sT[name][:, msl], sp[:])
                for n in range(NT):
                    nsl = slice(n * P, (n + 1) * P)
                    acc = ppsum.tile([P, 512], F32, name=f"acc_{name}{m}_{n}", tag="acc")
                    for k in range(NT):
                        nc.tensor.matmul(
                            acc[:], wreg[name][k][:, nsl], xc[:, k, :],
                            start=(k == 0), stop=False,
                        )
                    nc.tensor.matmul(
                        acc[:], lbt[:, noff + n * P : noff + (n + 1) * P],
                        sT[name][:, msl], start=False, stop=True,
                    )
                    nc.vector.tensor_scalar_add(
                        dest[n][:, msl], acc[:], bqk[:, bcol + n : bcol + n + 1]
                    )
                    if after_n is not None and m == MC - 1:
                        after_n(n)

        def emit_v_setup():
            nc.gpsimd.dma_start(bv_sb[:], bv_d.ap().partition_broadcast(P))
            for g in range(MS):
                va_cols = Va[g].rearrange("p (h c) -> p h c", c=VW)
                nc.vector.memset(va_cols[:, :, D], 1.0)

        def emit_proj_v(m):
            noff = 2 * E
            if True:
                msl = slice(m * 512, (m + 1) * 512)
                xc = xp.tile([P, NT, 512], F16, name=f"xc_v{m}", tag="xc")
                for k in range(NT):
                    nc.sync.dma_start(
                        xc[:, k, :], xT["v"].ap()[k * P : (k + 1) * P, msl]
                    )
                if m == 0:
                    for k in range(NT):
                        nc.sync.dma_start(
                            wreg["v"][k][:],
                            wT_d.ap()[k * P : (k + 1) * P, noff : noff + E],
                        )
                sp = spsum.tile([R, 512], F32, name=f"sp_v{m}", tag="sp")
                for k in range(NT):
                    nc.tensor.matmul(
                        sp[:], lat[:, k, :], xc[:, k, :],
                        start=(k == 0), stop=(k == NT - 1),
                    )
                nc.vector.tensor_copy(sT["v"][:, msl], sp[:])
                for ms_i in range(4):
                    g = m * 4 + ms_i
                    for nch in range(2):
                        ncols = 512 if nch == 0 else E - 512
                        nsl = slice(nch * 512, nch * 512 + ncols)
                        acc = ppsum.tile([P, 512], F32, name=f"accv{g}_{nch}", tag="acc")
                        for k in range(NT):
                            nc.tensor.matmul(
                                acc[:, :ncols],
                                xc[:, k, ms_i * P : (ms_i + 1) * P],
                                wreg["v"][k][:, nsl],
                                start=(k == 0), stop=False,
                            )
                        nc.tensor.matmul(
                            acc[:, :ncols],
                            sT["v"][:, g * P : (g + 1) * P],
                            lbt[:, noff + nch * 512 : noff + nch * 512 + ncols],
                            start=False, stop=True,
                        )
                        h0 = nch * 8
                        nh = 8 if nch == 0 else 4
                        for hi in range(nh):
                            h = h0 + hi
                            nc.vector.tensor_add(
                                Va[g][:, h * VW : h * VW + D],
                                acc[:, h * D - nch * 512 : (h + 1) * D - nch * 512],
                                bv_sb[:, h * D : (h + 1) * D],
                            )

        exps = {}

        def emit_scores(t):
            for j in range(JT):
                jsl = slice(j * P, (j + 1) * P)
                for hh in range(2):
                    base = hh * D
                    st = stp.tile([P, S], F32, name=f"st{t}_{j}_{hh}", tag="st")
                    for i in range(IC):
                        isl = slice(i * 512, (i + 1) * 512)
                        nc.tensor.matmul(
                            st[:, isl],
                            KT[t][base : base + D, jsl],
                            QT[t][base : base + D, isl],
                        )
                    ex = ep.tile([P, S], F16, name=f"ex{t}_{j}_{hh}", tag="ex")
                    nc.scalar.activation(
                        ex[:], st[:], mybir.ActivationFunctionType.Exp, bias=zbias[:]
                    )
                    exps[(t, hh, j)] = ex

        def emit_pv(t):
            zb = zbp.tile([P, S], F32, name=f"zb{t}", tag="zb")
            for i in range(IC):
                isl = slice(i * 512, (i + 1) * 512)
                zt = zsp.tile([2, 512], F16, name=f"zt{t}_{i}", tag="zt")
                for hh in range(2):
                    h = 2 * t + hh
                    base = hh * D
                    pv = ppsum.tile([P, 512], F32, name=f"pv{h}_{i}", tag="acc")
                    for j in range(JT):
                        nc.tensor.matmul(
                            pv[0:VW, :],
                            Va[j][:, h * VW : (h + 1) * VW],
                            exps[(t, hh, j)][:, isl],
                            start=(j == 0), stop=(j == JT - 1),
                        )
                    stage = sgp.tile([VW, 512], F16, name=f"stg{h}_{i}", tag="stg")
                    nc.vector.tensor_copy(stage[:], pv[0:VW, :])
                    nc.sync.dma_start(OTu[t][base : base + D, isl], stage[0:D, :])
                    nc.sync.dma_start(zt[hh : hh + 1, :], stage[D : D + 1, :])
                z32 = zsp.tile([2, 512], F32, name=f"z32_{t}_{i}", tag="z32")
                rz = zsp.tile([2, 512], F32, name=f"rz{t}_{i}", tag="rz")
                nc.vector.tensor_copy(z32[:], zt[:])
                nc.vector.reciprocal_approx_fast(rz[:], z32[:])
                nc.sync.dma_start(zdram[2 * t : 2 * t + 2, isl], rz[:])
                for hh in range(2):
                    nc.sync.dma_start(
                        zb[hh * D : (hh + 1) * D, isl],
                        zdram[2 * t + hh, isl].partition_broadcast(D),
                    )
                nc.vector.tensor_mul(OTn[t][:, isl], OTu[t][:, isl], zb[:, isl])

        # ---------------- emission sequence ----------------
        # q, then k (scores for pairs 0/1 fire as soon as their KT n-tile
        # lands), then v; the pair loop runs PV(t) against scores(t+2).
        emit_proj_qk("q")
        emit_proj_qk(
            "k", after_n=lambda n: emit_scores(n) if n < 2 else None
        )
        emit_v_setup()
        emit_proj_v(0)
        emit_proj_v(1)
        for t in range(NT):
            nc.sync.dma_start(woT[t][:], woT_d.ap()[t * P : (t + 1) * P, :])
        for t in range(NT):
            emit_pv(t)
            if t + 2 < NT:
                emit_scores(t + 2)
        wqk.release()
        dpool.release()
        zsp.release()
        zbp.release()
        sgp.release()
        ep.release()
        stp.release()
        xp.release()
        wpv.release()
        spsum.release()
        ppsum.release()

        # ---------------- Phase O: output projection ----------------
        with ExitStack() as octx:
            op = octx.enter_context(tc.tile_pool(name="op", bufs=4, space="PSUM"))
            fp = octx.enter_context(tc.tile_pool(name="fp", bufs=3))

            nc.gpsimd.dma_start(ob_sb[:], ob_d.ap().partition_broadcast(P))
            for m in range(MS):
                acc = op.tile([P, S], F32, name=f"oacc{m}", tag="oacc")
                for e in range(NT):
                    for nch in range(2):
                        ncols = 512 if nch == 0 else E - 512
                        nsl = slice(nch * 512, nch * 512 + ncols)
                        nc.tensor.matmul(
                            acc[:, nsl],
                            OTn[e][:, m * P : (m + 1) * P],
                            woT[e][:, nsl],
                            start=(e == 0),
                            stop=(e == NT - 1),
                        )
                fin = fp.tile([P, E], F32, name=f"fin{m}", tag="fin")
                nc.vector.tensor_add(fin[:], acc[:, :E], ob_sb[:])
                nc.sync.dma_start(out_d.ap()[m * P : (m + 1) * P, :], fin[:])

    nc.compile()
    return nc


def _prep_inputs(q, k, v, in_proj_weight, in_proj_bias, out_w, out_b, lora_a, lora_b):
    scale = float(D) ** -0.5
    q = np.asarray(q, np.float32)
    k = np.asarray(k, np.float32)
    v = np.asarray(v, np.float32)
    in_proj_weight = np.asarray(in_proj_weight, np.float32)
    in_proj_bias = np.asarray(in_proj_bias, np.float32)
    out_w = np.asarray(out_w, np.float32)
    out_b = np.asarray(out_b, np.float32)
    lora_a = np.asarray(lora_a, np.float32)
    lora_b = np.asarray(lora_b, np.float32)

    wT = in_proj_weight.T.copy()  # [E, 3E]
    wT[:, :E] *= scale
    lbT = lora_b.T.copy()  # [R, 3E]
    lbT[:, :E] *= scale
    bq = (in_proj_bias[:E] * scale).reshape(NT, P).T  # [P, NT]
    bk = in_proj_bias[E : 2 * E].reshape(NT, P).T
    bqk = np.ascontiguousarray(np.concatenate([bq, bk], axis=1), np.float32)

    shared = {
        "wT": np.ascontiguousarray(wT, np.float16),
        "woT": np.ascontiguousarray(out_w.T, np.float16),
        "laT": np.ascontiguousarray(lora_a.T, np.float16),
        "lbT": np.ascontiguousarray(lbT, np.float16),
        "bqk": bqk,
        "bv": np.ascontiguousarray(in_proj_bias[2 * E :], np.float32),
        "ob": np.ascontiguousarray(out_b, np.float32),
    }
    in_maps = []
    for b in range(8):
        m = dict(shared)
        m["xqT"] = np.ascontiguousarray(q[b].T, np.float16)
        m["xkT"] = np.ascontiguousarray(k[b].T, np.float16)
        m["xvT"] = np.ascontiguousarray(v[b].T, np.float16)
        in_maps.append(m)
    return in_maps


_NC_CACHE = {}


def run(inputs, trace=False, **spmd_kwargs):
    if "nc" not in _NC_CACHE:
        _NC_CACHE["nc"] = build_nc()
    nc = _NC_CACHE["nc"]
    in_maps = _prep_inputs(
        inputs["q"],
        inputs["k"],
        inputs["v"],
        inputs["in_proj_weight"],
        inputs["in_proj_bias"],
        inputs["out_w"],
        inputs["out_b"],
        inputs["lora_a"],
        inputs["lora_b"],
    )
    res = run_bass_kernel_spmd(
        nc, in_maps, core_ids=list(range(8)), trace=trace, **spmd_kwargs
    )
    out = np.stack([res.results[b]["out"] for b in range(8)]).astype(np.float32)
    return out, res


def kernel(
    q,
    k,
    v,
    in_proj_weight,
    in_proj_bias,
    out_w,
    out_b,
    lora_a,
    lora_b,
    num_heads=12,
    **_unused,
):
    assert int(num_heads) == H
    out, _ = run(
        {
            "q": q,
            "k": k,
            "v": v,
            "in_proj_weight": in_proj_weight,
            "in_proj_bias": in_proj_bias,
            "out_w": out_w,
            "out_b": out_b,
            "lora_a": lora_a,
            "lora_b": lora_b,
        }
    )
    return out


# revision 36
# speedup vs baseline: 1.0183x; 1.0183x over previous
"""Trainium2 Bass kernel for nn_Attention_48799418417201.

Multi-head attention (B=8, S=1024, E=768, H=12, D=64) with LoRA (R=16) on the
QKV projections. Data-parallel over batch: one batch element per NeuronCore,
8 cores.

Layout strategy (per core):
  - Host passes x^T [E, S] per input (q/k/v), plus pre-transposed weights, all
    fp16. The 1/sqrt(D) scaling is folded into Wq/bq/lora_b[q] on the host.
  - Projections produce Q^T, K^T [E, S] (head-major partitions) and V_aug
    [S, 13*65] (natural, 65 columns per head: 64 V columns + a ones column),
    each with the LoRA delta accumulated into the same PSUM group.
  - Scores are computed transposed: S^T[j, i] = sum_d K^T[d,j] Q^T[d,i], so
    softmax's sum runs over the partition axis -- the ones column in V_aug
    makes the PV matmul emit the softmax denominator Z into PSUM row 64 for
    free (M=65 streams the same cycles as M=64). exp() runs on ScalarE with
    no max-subtraction (scores are bounded ~[-2, 2] for these input scales).
  - PV produces O^T [E, S] directly (V is the stationary operand), which is
    exactly the layout the output projection needs as its stationary side;
    the kernel contains no on-device transposes at all.
  - Head-pair software pipeline: scores/exp for pair t overlap PV and the
    Z-reciprocal/normalize chain for pair t-1, so the output projection
    starts with no serial normalization tail.
"""

import numpy as np
from contextlib import ExitStack

import concourse.bass as bass
import concourse.bacc as bacc
import concourse.tile as tile
from concourse import mybir
from concourse.bass_utils import run_bass_kernel_spmd

P = 128
S = 1024  # sequence length
E = 768  # embedding
H = 12  # heads
D = 64  # head dim
R = 16  # lora rank
NT = E // P  # 6 n-tiles (also e-tiles) per 768-wide dim
MC = S // 512  # 2 moving-chunks of 512 along sequence
MS = S // P  # 8 sequence subtiles of 128
JT = S // P  # 8 j-tiles (key blocks)
IC = S // 512  # 2 i-chunks (query blocks of 512)
VW = D + 1  # 65 columns per head in V_aug

F16 = mybir.dt.float16
F32 = mybir.dt.float32


def build_nc():
    nc = bacc.Bacc("TRN2", target_bir_lowering=False, debug=False, num_devices=8)

    xT = {
        name: nc.dram_tensor(f"x{name}T", [E, S], F16, kind="ExternalInput")
        for name in ("q", "k", "v")
    }
    wT_d = nc.dram_tensor("wT", [E, 3 * E], F16, kind="ExternalInput")
    woT_d = nc.dram_tensor("woT", [E, E], F16, kind="ExternalInput")
    laT_d = nc.dram_tensor("laT", [E, R], F16, kind="ExternalInput")
    lbT_d = nc.dram_tensor("lbT", [R, 3 * E], F16, kind="ExternalInput")
    bqk_d = nc.dram_tensor("bqk", [P, 2 * NT], F32, kind="ExternalInput")
    bv_d = nc.dram_tensor("bv", [E], F32, kind="ExternalInput")
    ob_d = nc.dram_tensor("ob", [E], F32, kind="ExternalInput")
    out_d = nc.dram_tensor("out", [S, E], F32, kind="ExternalOutput")

    with tile.TileContext(nc) as tc, ExitStack() as perm:
        pp = perm.enter_context(tc.tile_pool(name="perm", bufs=1))

        QT = [pp.tile([P, S], F16, name=f"QT{t}", tag=f"QT{t}") for t in range(NT)]
        KT = [pp.tile([P, S], F16, name=f"KT{t}", tag=f"KT{t}") for t in range(NT)]
        Va = [pp.tile([P, H * VW], F16, name=f"Va{m}", tag=f"Va{m}") for m in range(MS)]
        OTu = [pp.tile([P, S], F16, name=f"OTu{t}", tag=f"OTu{t}") for t in range(NT)]
        OTn = [pp.tile([P, S], F16, name=f"OTn{t}", tag=f"OTn{t}") for t in range(NT)]

        sT = {
            n: pp.tile([R, S], F16, name=f"sT{n}", tag=f"sT{n}")
            for n in ("q", "k", "v")
        }
        woT = [pp.tile([P, E], F16, name=f"woT{t}", tag=f"woT{t}") for t in range(NT)]
        bqk = pp.tile([P, 2 * NT], F32, name="bqk", tag="bqk")
        bv_sb = pp.tile([P, E], F32, name="bv_sb", tag="bv_sb")
        ob_sb = pp.tile([P, E], F32, name="ob_sb", tag="ob_sb")
        zbias = pp.tile([P, 1], F32, name="zbias", tag="zbias")

        nc.vector.memset(zbias[:], 0.0)
        nc.sync.dma_start(bqk[:], bqk_d.ap()[:])

        # ---------------- pools ----------------
        # PSUM bank budget: qk-proj {ppsum 3 + spsum 1}; overlap window adds
        # stp (2x2 banks) = 8; after v-proj ppsum/spsum release -> pvp 2;
        # output projection uses op 4x2 banks alone.
        ppsum = tc.alloc_tile_pool(name="ppsum", bufs=3, space="PSUM")
        spsum = tc.alloc_tile_pool(name="spsum", bufs=1, space="PSUM")
        wpv = tc.alloc_tile_pool(name="wpv", bufs=1)
        xp = tc.alloc_tile_pool(name="xp", bufs=2)
        stp = tc.alloc_tile_pool(name="stp", bufs=2, space="PSUM")
        ep = tc.alloc_tile_pool(name="ep", bufs=26)
        sgp = tc.alloc_tile_pool(name="sgp", bufs=2)
        zbp = tc.alloc_tile_pool(name="zbp", bufs=2)
        zsp = tc.alloc_tile_pool(name="zsp", bufs=1)
        dpool = tc.alloc_tile_pool(name="dpool", bufs=1, space="DRAM")
        wqk = tc.alloc_tile_pool(name="wqk", bufs=1)
        zdram = dpool.tile([H, S], F32, name="zdram", tag="zdram")

        lat = wpv.tile([P, NT, R], F16, name="lat", tag="lat")
        lbt = wpv.tile([R, 3 * E], F16, name="lbt", tag="lbt")
        for k in range(NT):
            nc.sync.dma_start(lat[:, k, :], laT_d.ap()[k * P : (k + 1) * P, :])
        nc.sync.dma_start(lbt[:], lbT_d.ap()[:])
        wreg = {}
        for name in ("q", "k"):
            wreg[name] = [
                wqk.tile([P, E], F16, name=f"w{name}{k}", tag=f"w{name}{k}")
                for k in range(NT)
            ]
        wreg["v"] = [
            wpv.tile([P, E], F16, name=f"wv{k}", tag=f"wv{k}") for k in range(NT)
        ]

        def emit_proj_qk(name, after_n=None):
            noff = (0 if name == "q" else E)
            dest = QT if name == "q" else KT
            bcol = 0 if name == "q" else NT
            for m in range(MC):
                msl = slice(m * 512, (m + 1) * 512)
                xc = xp.tile([P, NT, 512], F16, name=f"xc_{name}{m}", tag="xc")
                for k in range(NT):
                    nc.sync.dma_start(
                        xc[:, k, :], xT[name].ap()[k * P : (k + 1) * P, msl]
                    )
                if m == 0:
                    for k in range(NT):
                        nc.sync.dma_start(
                            wreg[name][k][:],
                            wT_d.ap()[k * P : (k + 1) * P, noff : noff + E],
                        )
                sp = spsum.tile([R, 512], F32, name=f"sp_{name}{m}", tag="sp")
                for k in range(NT):
                    nc.tensor.matmul(
                        sp[:], lat[:, k, :], xc[:, k, :],
                        start=(k == 0), stop=(k == NT - 1),
                    )
                nc.vector.tensor_copy(sT[name][:, msl], sp[:])
                for n in range(NT):
                    nsl = slice(n * P, (n + 1) * P)
                    acc = ppsum.tile([P, 512], F32, name=f"acc_{name}{m}_{n}", tag="acc")
                    for k in range(NT):
                        nc.tensor.matmul(
                            acc[:], wreg[name][k][:, nsl], xc[:, k, :],
                            start=(k == 0), stop=False,
                        )
                    nc.tensor.matmul(
                        acc[:], lbt[:, noff + n * P : noff + (n + 1) * P],
                        sT[name][:, msl], start=False, stop=True,
                    )
                    nc.vector.tensor_scalar_add(
                        dest[n][:, msl], acc[:], bqk[:, bcol + n : bcol + n + 1]
                    )
                    if after_n is not None and m == MC - 1:
                        after_n(n)

        def emit_v_setup():
            nc.sync.dma_start(bv_sb[:], bv_d.ap().partition_broadcast(P))
            for g in range(MS):
                va_cols = Va[g].rearrange("p (h c) -> p h c", c=VW)
                nc.vector.memset(va_cols[:, :, D], 1.0)

        def emit_proj_v(m):
            noff = 2 * E
            if True:
                msl = slice(m * 512, (m + 1) * 512)
                xc = xp.tile([P, NT, 512], F16, name=f"xc_v{m}", tag="xc")
                for k in range(NT):
                    nc.sync.dma_start(
                        xc[:, k, :], xT["v"].ap()[k * P : (k + 1) * P, msl]
                    )
                if m == 0:
                    for k in range(NT):
                        nc.sync.dma_start(
                            wreg["v"][k][:],
                            wT_d.ap()[k * P : (k + 1) * P, noff : noff + E],
                        )
                sp = spsum.tile([R, 512], F32, name=f"sp_v{m}", tag="sp")
                for k in range(NT):
                    nc.tensor.matmul(
                        sp[:], lat[:, k, :], xc[:, k, :],
                        start=(k == 0), stop=(k == NT - 1),
                    )
                nc.vector.tensor_copy(sT["v"][:, msl], sp[:])
                for ms_i in range(4):
                    g = m * 4 + ms_i
                    for nch in range(2):
                        ncols = 512 if nch == 0 else E - 512
                        nsl = slice(nch * 512, nch * 512 + ncols)
                        acc = ppsum.tile([P, 512], F32, name=f"accv{g}_{nch}", tag="acc")
                        for k in range(NT):
                            nc.tensor.matmul(
                                acc[:, :ncols],
                                xc[:, k, ms_i * P : (ms_i + 1) * P],
                                wreg["v"][k][:, nsl],
                                start=(k == 0), stop=False,
                            )
                        nc.tensor.matmul(
                            acc[:, :ncols],
                            sT["v"][:, g * P : (g + 1) * P],
                            lbt[:, noff + nch * 512 : noff + nch * 512 + ncols],
                            start=False, stop=True,
                        )
                        h0 = nch * 8
                        nh = 8 if nch == 0 else 4
                        for hi in range(nh):
                            h = h0 + hi
                            nc.vector.tensor_add(
                                Va[g][:, h * VW : h * VW + D],
                                acc[:, h * D - nch * 512 : (h + 1) * D - nch * 512],
                                bv_sb[:, h * D : (h + 1) * D],
                            )

        exps = {}

        def emit_scores(t):
            for j in range(JT):
                jsl = slice(j * P, (j + 1) * P)
                for hh in range(2):
                    base = hh * D
                    st = stp.tile([P, S], F32, name=f"st{t}_{j}_{hh}", tag="st")
                    for i in range(IC):
                        isl = slice(i * 512, (i + 1) * 512)
                        nc.tensor.matmul(
                            st[:, isl],
                            KT[t][base : base + D, jsl],
                            QT[t][base : base + D, isl],
                        )
                    ex = ep.tile([P, S], F16, name=f"ex{t}_{j}_{hh}", tag="ex")
                    nc.scalar.activation(
                        ex[:], st[:], mybir.ActivationFunctionType.Exp, bias=zbias[:]
                    )
                    exps[(t, hh, j)] = ex

        def emit_pv(t):
            zb = zbp.tile([P, S], F32, name=f"zb{t}", tag="zb")
            for i in range(IC):
                isl = slice(i * 512, (i + 1) * 512)
                zt = zsp.tile([2, 512], F16, name=f"zt{t}_{i}", tag="zt")
                for hh in range(2):
                    h = 2 * t + hh
                    base = hh * D
                    pv = ppsum.tile([P, 512], F32, name=f"pv{h}_{i}", tag="acc")
                    for j in range(JT):
                        nc.tensor.matmul(
                            pv[0:VW, :],
                            Va[j][:, h * VW : (h + 1) * VW],
                            exps[(t, hh, j)][:, isl],
                            start=(j == 0), stop=(j == JT - 1),
                        )
                    stage = sgp.tile([VW, 512], F16, name=f"stg{h}_{i}", tag="stg")
                    nc.vector.tensor_copy(stage[:], pv[0:VW, :])
                    nc.sync.dma_start(OTu[t][base : base + D, isl], stage[0:D, :])
                    nc.sync.dma_start(zt[hh : hh + 1, :], stage[D : D + 1, :])
                z32 = zsp.tile([2, 512], F32, name=f"z32_{t}_{i}", tag="z32")
                rz = zsp.tile([2, 512], F32, name=f"rz{t}_{i}", tag="rz")
                nc.vector.tensor_copy(z32[:], zt[:])
                nc.vector.reciprocal_approx_fast(rz[:], z32[:])
                nc.sync.dma_start(zdram[2 * t : 2 * t + 2, isl], rz[:])
                for hh in range(2):
                    nc.sync.dma_start(
                        zb[hh * D : (hh + 1) * D, isl],
                        zdram[2 * t + hh, isl].partition_broadcast(D),
                    )
                nc.vector.tensor_mul(OTn[t][:, isl], OTu[t][:, isl], zb[:, isl])

        # ---------------- emission sequence ----------------
        # q, then k (scores for pairs 0/1 fire as soon as their KT n-tile
        # lands), then v; the pair loop runs PV(t) against scores(t+2).
        emit_proj_qk("q")
        emit_proj_qk(
            "k", after_n=lambda n: emit_scores(n) if n < 2 else None
        )
        emit_v_setup()
        emit_proj_v(0)
        emit_proj_v(1)
        for t in range(NT):
            nc.sync.dma_start(woT[t][:], woT_d.ap()[t * P : (t + 1) * P, :])
        for t in range(NT):
            emit_pv(t)
            if t + 2 < NT:
                emit_scores(t + 2)
        wqk.release()
        dpool.release()
        zsp.release()
        zbp.release()
        sgp.release()
        ep.release()
        stp.release()
        xp.release()
        wpv.release()
        spsum.release()
        ppsum.release()

        # ---------------- Phase O: output projection ----------------
        with ExitStack() as octx:
            op = octx.enter_context(tc.tile_pool(name="op", bufs=4, space="PSUM"))
            fp = octx.enter_context(tc.tile_pool(name="fp", bufs=3))

            nc.sync.dma_start(ob_sb[:], ob_d.ap().partition_broadcast(P))
            for m in range(MS):
                acc = op.tile([P, S], F32, name=f"oacc{m}", tag="oacc")
                for e in range(NT):
                    for nch in range(2):
                        ncols = 512 if nch == 0 else E - 512
                        nsl = slice(nch * 512, nch * 512 + ncols)
                        nc.tensor.matmul(
                            acc[:, nsl],
                            OTn[e][:, m * P : (m + 1) * P],
                            woT[e][:, nsl],
                            start=(e == 0),
                            stop=(e == NT - 1),
                        )
                fin = fp.tile([P, E], F32, name=f"fin{m}", tag="fin")
                nc.vector.tensor_add(fin[:], acc[:, :E], ob_sb[:])
                nc.sync.dma_start(out_d.ap()[m * P : (m + 1) * P, :], fin[:])

    nc.compile()
    return nc


def _prep_inputs(q, k, v, in_proj_weight, in_proj_bias, out_w, out_b, lora_a, lora_b):
    scale = float(D) ** -0.5
    q = np.asarray(q, np.float32)
    k = np.asarray(k, np.float32)
    v = np.asarray(v, np.float32)
    in_proj_weight = np.asarray(in_proj_weight, np.float32)
    in_proj_bias = np.asarray(in_proj_bias, np.float32)
    out_w = np.asarray(out_w, np.float32)
    out_b = np.asarray(out_b, np.float32)
    lora_a = np.asarray(lora_a, np.float32)
    lora_b = np.asarray(lora_b, np.float32)

    wT = in_proj_weight.T.copy()  # [E, 3E]
    wT[:, :E] *= scale
    lbT = lora_b.T.copy()  # [R, 3E]
    lbT[:, :E] *= scale
    bq = (in_proj_bias[:E] * scale).reshape(NT, P).T  # [P, NT]
    bk = in_proj_bias[E : 2 * E].reshape(NT, P).T
    bqk = np.ascontiguousarray(np.concatenate([bq, bk], axis=1), np.float32)

    shared = {
        "wT": np.ascontiguousarray(wT, np.float16),
        "woT": np.ascontiguousarray(out_w.T, np.float16),
        "laT": np.ascontiguousarray(lora_a.T, np.float16),
        "lbT": np.ascontiguousarray(lbT, np.float16),
        "bqk": bqk,
        "bv": np.ascontiguousarray(in_proj_bias[2 * E :], np.float32),
        "ob": np.ascontiguousarray(out_b, np.float32),
    }
    in_maps = []
    for b in range(8):
        m = dict(shared)
        m["xqT"] = np.ascontiguousarray(q[b].T, np.float16)
        m["xkT"] = np.ascontiguousarray(k[b].T, np.float16)
        m["xvT"] = np.ascontiguousarray(v[b].T, np.float16)
        in_maps.append(m)
    return in_maps


_NC_CACHE = {}


def run(inputs, trace=False, **spmd_kwargs):
    if "nc" not in _NC_CACHE:
        _NC_CACHE["nc"] = build_nc()
    nc = _NC_CACHE["nc"]
    in_maps = _prep_inputs(
        inputs["q"],
        inputs["k"],
        inputs["v"],
        inputs["in_proj_weight"],
        inputs["in_proj_bias"],
        inputs["out_w"],
        inputs["out_b"],
        inputs["lora_a"],
        inputs["lora_b"],
    )
    res = run_bass_kernel_spmd(
        nc, in_maps, core_ids=list(range(8)), trace=trace, **spmd_kwargs
    )
    out = np.stack([res.results[b]["out"] for b in range(8)]).astype(np.float32)
    return out, res


def kernel(
    q,
    k,
    v,
    in_proj_weight,
    in_proj_bias,
    out_w,
    out_b,
    lora_a,
    lora_b,
    num_heads=12,
    **_unused,
):
    assert int(num_heads) == H
    out, _ = run(
        {
            "q": q,
            "k": k,
            "v": v,
            "in_proj_weight": in_proj_weight,
            "in_proj_bias": in_proj_bias,
            "out_w": out_w,
            "out_b": out_b,
            "lora_a": lora_a,
            "lora_b": lora_b,
        }
    )
    return out
